# revision 26
# baseline (speedup 1.0000x reference)
"""Multi-head attention forward for Trainium2, 8 NeuronCores.

Problem: B=4, S=2048, D=1024, H=16 heads (dk=64), fp32 reference:
  q/k/v = x @ W{q,k,v}^T + b ; heads split; softmax(q k^T / 8) v ; out @ Wo^T + bo

Sharding: 8 cores = 4 batches x 2 head-groups (8 heads each), Megatron-style:
each core computes its batch's attention for its 8 heads plus the partial
output projection (Wo column slice); host sums the two partials per batch.

Per-core schedule (ACT-engine exp is the floor; keep it saturated):
  V = x@WvT+bv upfront (overlapped with the x DMA stream), then A(0)
  (Q^T/K^T for head-pair 0), then for each pair p: 4 blocks of 512
  queries; per key-tile: 2 row-tiled score matmuls -> one wide exp over
  both heads -> 2 col-tiled PV matmuls + 2 col-tiled ones64-denominator
  matmuls (denominator lands replicated across the same 64 partitions
  as each head's O, so normalization is one reciprocal + one multiply
  on DVE, no replication DMAs). Scores run 2 key-tiles ahead through 3
  rotating PSUM buffers and PV/den lag one step, so the in-order PE
  never head-of-line blocks and the ACT engine chains exps
  back-to-back. A(p+1) chains and (for the last pair)
  output-projection chains are pumped as bursts into the PE slack of
  the ACT-bound blocks via generators, borrowing score-pool slots;
  Tile dependency tracking keeps it correct. y = O^T @ WoT + bo drains
  at the end.

  Scheduling facts measured on HW (slope bench): tile_position row/col
  packed matmul pairs run fully concurrent (~208ns/pair at free=512);
  exp([128,1024]) cadence ~1.1us incl. consumer coupling; ~500ns
  sem/ack latency is why scores need two buffers of lead. All matmul
  inputs are fp16 (rel err ~6e-4 vs the fp32 reference, tol 2e-2).
"""

import sys

sys.path.insert(0, "/opt/trn_rl_repo")

import numpy as np

import concourse.bass as bass  # noqa: F401
import concourse.mybir as mybir
import concourse.tile as tile
from concourse import bacc, bass_utils

B, S, D, H = 4, 2048, 1024, 16
DK = D // H          # 64
G = 2                # head groups (tensor-parallel factor)
DL = D // G          # 512 local features per core
NPAIR = DL // 128    # 4 head-pairs per core
EC = D // 128        # 8 contraction chunks for projections
ST = S // 128        # 16 s-tiles
KT = S // 128        # 16 key tiles
QB = S // 512        # 4 query blocks of 512

F32R = mybir.dt.float32r
F32 = mybir.dt.float32
F16 = mybir.dt.float16

_CACHED = {}


def _build_nc(loop_n=1):
    nc = bacc.Bacc(None, target_bir_lowering=False)

    xT = nc.dram_tensor("xT", [D, S], F16, kind="ExternalInput")
    wqT = nc.dram_tensor("wqT", [D, DL], F16, kind="ExternalInput")
    wkT = nc.dram_tensor("wkT", [D, DL], F16, kind="ExternalInput")
    wvT = nc.dram_tensor("wvT", [D, DL], F16, kind="ExternalInput")
    woT = nc.dram_tensor("woT", [DL, D], F16, kind="ExternalInput")
    bq = nc.dram_tensor("bq", [DL], F32, kind="ExternalInput")
    bk = nc.dram_tensor("bk", [DL], F32, kind="ExternalInput")
    bv = nc.dram_tensor("bv", [1, DL], F16, kind="ExternalInput")
    bo = nc.dram_tensor("bo", [1, D], F16, kind="ExternalInput")
    y = nc.dram_tensor("y", [S, D], F16, kind="ExternalOutput")

    with tile.TileContext(nc) as tc:
      # Pools wrap the on-device repeat loop so consecutive reps pipeline:
      # rep N+1's input DMAs and V/A chains overlap rep N's tail instead
      # of waiting on pool-closure barriers.
      with (
            tc.tile_pool(name="main", bufs=1) as pmain,
            tc.tile_pool(name="xw", bufs=1) as pxw,
            tc.tile_pool(name="wqk", bufs=2) as pwqk,
            tc.tile_pool(name="qkt", bufs=2) as pqkt,
            tc.tile_pool(name="ptile", bufs=3) as ppt,
            tc.tile_pool(name="rtile", bufs=2) as prt,
            tc.tile_pool(name="ytile", bufs=3) as pyt,
            tc.tile_pool(name="pstp", bufs=3, space="PSUM") as pstp,
            tc.tile_pool(name="potp", bufs=1, space="PSUM") as potp,
            tc.tile_pool(name="pdnp", bufs=1, space="PSUM") as pdnp,
      ):
        for _rep in range(loop_n):
            # persistent tiles
            vt = pmain.tile([128, ST, DL], F16, tag="vt")
            ot = pmain.tile([128, NPAIR, S], F16, tag="ot")
            ones64 = pmain.tile([128, 64], F16, tag="ones64")
            ones1h = pmain.tile([1, 128], F16, tag="ones1h")
            bqt = pmain.tile([128, NPAIR], F32, tag="bqt")
            bkt = pmain.tile([128, NPAIR], F32, tag="bkt")
            bvt = pmain.tile([1, DL], F16, tag="bvt")
            bot = pmain.tile([1, D], F16, tag="bot")
            wot = pmain.tile([128, NPAIR, D], F16, tag="wot")

            nc.vector.memset(ones64[:], 1.0)
            nc.vector.memset(ones1h[:], 1.0)
            nc.sync.dma_start(bqt[:], bq.ap().rearrange("(p d) -> d p", d=128))
            nc.sync.dma_start(bkt[:], bk.ap().rearrange("(p d) -> d p", d=128))
            nc.sync.dma_start(bvt[:], bv.ap())
            nc.sync.dma_start(bot[:], bo.ap())
            for dc in range(NPAIR):
                nc.sync.dma_start(wot[:, dc], woT.ap()[dc * 128:(dc + 1) * 128, :])

            # x / Wv stream in ec order so phase V can start on chunk 0
            xt = pxw.tile([128, EC, S], F16, tag="xt")
            wvt = pxw.tile([128, EC, DL], F16, tag="wvt")
            for ec in range(EC):
                nc.sync.dma_start(wvt[:, ec], wvT.ap()[ec * 128:(ec + 1) * 128, :])
                nc.sync.dma_start(xt[:, ec], xT.ap()[ec * 128:(ec + 1) * 128, :])

            # All projection chains (V, A, C) borrow score-pool slots (tag
            # "stt", half-used): PSUM stays at 8 banks (3x2 stt + otp +
            # dnp) while still letting A/C matmuls pump into B's PE slack.
            def chain_ps():
                t = pstp.tile([128, 1024], F32, tag="stt")
                return t

            # ---- V = x @ WvT + bv  (natural [s, d] layout, fp16)
            for st in range(ST):
                vps = chain_ps()
                for ec in range(EC):
                    nc.tensor.matmul(
                        vps[:, 0:DL], xt[:, ec, st * 128:(st + 1) * 128],
                        wvt[:, ec], start=(ec == 0), stop=False)
                nc.tensor.matmul(vps[:, 0:DL], ones1h[:], bvt[:],
                                 start=False, stop=True)
                nc.vector.tensor_copy(vt[:, st], vps[:, 0:DL])

            qts = [None] * NPAIR
            kts = [None] * NPAIR

            def phase_a_gen(p):
                wqp = pwqk.tile([128, EC, 128], F16, tag="wqp")
                wkp = pwqk.tile([128, EC, 128], F16, tag="wkp")
                for ec in range(EC):
                    nc.sync.dma_start(
                        wqp[:, ec],
                        wqT.ap()[ec * 128:(ec + 1) * 128, p * 128:(p + 1) * 128])
                    nc.sync.dma_start(
                        wkp[:, ec],
                        wkT.ap()[ec * 128:(ec + 1) * 128, p * 128:(p + 1) * 128])
                qt = pqkt.tile([128, S], F16, tag="qt")
                kt = pqkt.tile([128, S], F16, tag="kt")
                yield
                # one yield per full chain burst: the chain's stt-pool slot
                # is held only ~2.1us, which the lead-2 exp queue rides out
                for dst, wp, bias in ((qt, wqp, bqt), (kt, wkp, bkt)):
                    for qc in range(4):
                        ps = chain_ps()
                        for ec in range(EC):
                            nc.tensor.matmul(
                                ps[:, 0:512], wp[:, ec],
                                xt[:, ec, qc * 512:(qc + 1) * 512],
                                start=(ec == 0), stop=(ec == EC - 1))
                        nc.vector.tensor_scalar_add(
                            dst[:, qc * 512:(qc + 1) * 512], ps[:, 0:512],
                            bias[:, p:p + 1])
                        yield
                qts[p], kts[p] = qt, kt

            def phase_c_gen():
                for st in range(ST):
                    ss = slice(st * 128, (st + 1) * 128)
                    for e2 in range(2):
                        es = slice(e2 * 512, (e2 + 1) * 512)
                        yps = chain_ps()
                        for dc in range(NPAIR):
                            nc.tensor.matmul(
                                yps[:, 0:512], ot[:, dc, ss], wot[:, dc, es],
                                start=(dc == 0), stop=False)
                        nc.tensor.matmul(
                            yps[:, 0:512], ones1h[:], bot[:, es],
                            start=False, stop=True)
                        yt = pyt.tile([128, 512], F16, tag="yt")
                        nc.vector.tensor_copy(yt[:], yps[:, 0:512])
                        nc.sync.dma_start(y.ap()[ss, es], yt[:])
                        yield

            def pump(gen, n):
                if gen is None:
                    return None
                for _ in range(n):
                    try:
                        next(gen)
                    except StopIteration:
                        return None
                return gen

            # A(0) upfront
            a0 = phase_a_gen(0)
            while pump(a0, 16) is not None:
                pass

            cgen = phase_c_gen()
            c_done = 0          # yields taken from cgen

            for p in range(NPAIR):
                agen = phase_a_gen(p + 1) if p + 1 < NPAIR else None
                qt, kt = qts[p], kts[p]
                for qb in range(QB):
                    qs = slice(qb * 512, (qb + 1) * 512)
                    otp = potp.tile([128, 512], F32, tag="otp")
                    dnp = pdnp.tile([128, 512], F32, tag="dnp")
                    # C(st) is safe once B(3, st//4) is fully drained; cap
                    # the pump budget (in chains) to tokens covered by
                    # finished blocks.
                    c_budget = 8 * qb if p == NPAIR - 1 else 0

                    def issue_scores(kti):
                        ks = slice(kti * 128, (kti + 1) * 128)
                        stt = pstp.tile([128, 1024], F32, tag="stt")
                        nc.tensor.matmul(
                            stt[:, 0:512], kt[0:64, ks], qt[0:64, qs],
                            start=True, stop=True, tile_position=(0, 0))
                        nc.tensor.matmul(
                            stt[:, 512:1024], kt[64:128, ks], qt[64:128, qs],
                            start=True, stop=True, tile_position=(64, 0))
                        return stt

                    pL = p * 128

                    def issue_pvden(kti, pt):
                        first, last = kti == 0, kti == KT - 1
                        nc.tensor.matmul(
                            otp[0:64, :], vt[:, kti, pL:pL + 64], pt[:, 0:512],
                            start=first, stop=last, tile_position=(0, 0))
                        nc.tensor.matmul(
                            otp[64:128, :], vt[:, kti, pL + 64:pL + 128],
                            pt[:, 512:1024],
                            start=first, stop=last, tile_position=(0, 64))
                        nc.tensor.matmul(
                            dnp[0:64, :], ones64[:], pt[:, 0:512],
                            start=first, stop=last, tile_position=(0, 0))
                        nc.tensor.matmul(
                            dnp[64:128, :], ones64[:], pt[:, 512:1024],
                            start=first, stop=last, tile_position=(0, 64))

                    # Software pipeline, scores 2 ahead (3 stt bufs) and
                    # PV/den lagged one step: per iter issue exp(k) ->
                    # scores(k+2) -> pumped A/C matmuls -> PV/den(k-1).
                    # The ~500ns sem/ack latency needs two buffers of
                    # lead for the ACT engine to chain exps back-to-back.
                    stts = [issue_scores(0), issue_scores(1)]
                    prev = None
                    for kti in range(KT):
                        pt = ppt.tile([128, 1024], F16, tag="pt")
                        nc.scalar.activation(
                            pt[:], stts[kti][:],
                            mybir.ActivationFunctionType.Exp, scale=0.125)
                        if kti + 2 < KT:
                            stts.append(issue_scores(kti + 2))
                        if kti % 4 == 2:
                            agen = pump(agen, 1)
                        if c_budget > c_done and kti >= 1 and kti % 2 == 1:
                            if pump(cgen, 1) is None:
                                cgen = None
                            c_done += 1
                        if prev is not None:
                            issue_pvden(*prev)
                        prev = (kti, pt)
                    issue_pvden(*prev)
                    rt = prt.tile([128, 512], F32, tag="rt")
                    nc.vector.reciprocal(rt[:], dnp[:])
                    nc.vector.tensor_mul(ot[:, p, qs], otp[:], rt[:])
                while pump(agen, 16) is not None:
                    pass

            # ---- C: drain y = OT^T @ WoT + bo
            while cgen is not None and pump(cgen, 16) is not None:
                pass

    nc.compile()
    return nc


def _get_nc(loop_n=1):
    key = f"nc{loop_n}"
    if key not in _CACHED:
        _CACHED[key] = _build_nc(loop_n)
    return _CACHED[key]


def _get_runner():
    """Build the 8-core SPMD executable once and cache it, so repeated
    kernel() calls skip jax re-tracing and NEFF compilation."""
    if "runner" in _CACHED:
        return _CACHED["runner"]

    import jax
    from jax.sharding import Mesh, NamedSharding, PartitionSpec
    from jax.experimental.shard_map import shard_map
    from concourse import bass2jax
    from concourse.bass2jax import _bass_exec_p, install_neuronx_cc_hook

    nc = _get_nc()
    install_neuronx_cc_hook()
    partition_name = nc.partition_id_tensor.name if nc.partition_id_tensor else None
    in_names, out_names, out_avals, zero_outs = [], [], [], []
    for alloc in nc.m.functions[0].allocations:
        if not isinstance(alloc, mybir.MemoryLocationSet):
            continue
        name = alloc.memorylocations[0].name
        if alloc.kind == "ExternalInput":
            if name != partition_name:
                in_names.append(name)
        elif alloc.kind == "ExternalOutput":
            out_names.append(name)
            shape = tuple(alloc.tensor_shape)
            dtype = mybir.dt.np(alloc.dtype)
            out_avals.append(jax.core.ShapedArray(shape, dtype))
            zero_outs.append(np.zeros(shape, dtype))
    n_params, n_outs = len(in_names), len(out_avals)
    all_names = in_names + out_names + ([partition_name] if partition_name else [])

    def _body(*args):
        operands = list(args)
        if partition_name is not None:
            operands.append(bass2jax.partition_id_tensor())
        outs = _bass_exec_p.bind(
            *operands,
            out_avals=tuple(out_avals),
            in_names=tuple(all_names),
            out_names=tuple(out_names),
            lowering_input_output_aliases=(),
            sim_require_finite=True,
            sim_require_nnan=True,
            nc=nc,
        )
        return tuple(outs)

    devices = jax.devices()[:8]
    mesh = Mesh(np.asarray(devices), ("core",))
    f = jax.jit(
        shard_map(
            _body, mesh=mesh,
            in_specs=(PartitionSpec("core"),) * (n_params + n_outs),
            out_specs=(PartitionSpec("core"),) * n_outs,
            check_rep=False,
        ),
        donate_argnums=tuple(range(n_params, n_params + n_outs)),
        keep_unused=True,
    )
    shard = NamedSharding(mesh, PartitionSpec("core"))
    state = {
        "f": f, "in_names": in_names, "out_names": out_names,
        "zero_outs": zero_outs, "shard": shard, "jax": jax, "last_outs": None,
    }
    _CACHED["runner"] = state
    return state


def make_in_maps(encoder_input, Wq_w, Wq_b, Wk_w, Wk_b, Wv_w, Wv_b, Wo_w, Wo_b):
    encoder_input = np.asarray(encoder_input, dtype=np.float32)
    Wq_w = np.asarray(Wq_w, dtype=np.float32)
    Wk_w = np.asarray(Wk_w, dtype=np.float32)
    Wv_w = np.asarray(Wv_w, dtype=np.float32)
    Wo_w = np.asarray(Wo_w, dtype=np.float32)
    Wq_b = np.asarray(Wq_b, dtype=np.float32)
    Wk_b = np.asarray(Wk_b, dtype=np.float32)
    Wv_b = np.asarray(Wv_b, dtype=np.float32)
    Wo_b = np.asarray(Wo_b, dtype=np.float32)

    xT_f16, woT_f16 = {}, {}
    in_maps = []
    for core in range(8):
        b, g = divmod(core, G)
        gs = slice(g * DL, (g + 1) * DL)
        if b not in xT_f16:
            xT_f16[b] = np.ascontiguousarray(encoder_input[b].T).astype(np.float16)
        if g not in woT_f16:
            woT_f16[g] = np.ascontiguousarray(Wo_w[:, gs].T).astype(np.float16)
        in_maps.append({
            "xT": xT_f16[b],
            "wqT": np.ascontiguousarray(Wq_w[gs, :].T).astype(np.float16),
            "wkT": np.ascontiguousarray(Wk_w[gs, :].T).astype(np.float16),
            "wvT": np.ascontiguousarray(Wv_w[gs, :].T).astype(np.float16),
            "woT": woT_f16[g],
            "bq": np.ascontiguousarray(Wq_b[gs]),
            "bk": np.ascontiguousarray(Wk_b[gs]),
            "bv": Wv_b[gs].astype(np.float16).reshape(1, DL),
            "bo": (Wo_b if g == 0 else np.zeros_like(Wo_b))
                  .astype(np.float16).reshape(1, D),
        })
    return in_maps


def kernel(encoder_input, attention_mask, Wq_w, Wq_b, Wk_w, Wk_b, Wv_w, Wv_b,
           Wo_w, Wo_b):
    del attention_mask  # dead input in the reference forward
    in_maps = make_in_maps(encoder_input, Wq_w, Wq_b, Wk_w, Wk_b, Wv_w, Wv_b,
                           Wo_w, Wo_b)
    r = _get_runner()
    jax = r["jax"]

    concat_in = [
        jax.device_put(
            np.concatenate([in_maps[c][n] for c in range(8)], axis=0), r["shard"])
        for n in r["in_names"]
    ]
    outs = r["last_outs"]
    if outs is None:
        outs = [
            jax.device_put(
                np.zeros((8 * z.shape[0], *z.shape[1:]), z.dtype), r["shard"])
            for z in r["zero_outs"]
        ]
    outs = r["f"](*concat_in, *outs)
    np_outs = [np.asarray(o) for o in outs]
    # keep the returned device buffers to donate on the next call
    r["last_outs"] = list(outs)

    per_core = {}
    for i, nme in enumerate(r["out_names"]):
        full = np_outs[i].reshape(8, -1, *np_outs[i].shape[1:])
        per_core[nme] = full

    y = per_core["y"]
    out = np.empty((B, S, D), dtype=np.float32)
    for b in range(B):
        out[b] = y[G * b].astype(np.float32) + y[G * b + 1].astype(np.float32)
    return out
